# revision 28
# baseline (speedup 1.0000x reference)
"""Low-rank self-attention on 8 trn2 NeuronCores.

reference math (per batch b):
  q = x @ Wq.T            [S,R]
  k = x @ Wk.T            [S,R]
  P = softmax(q k^T / sqrt(R))    (mask is all-ones -> no-op)
  out = (P (x @ Wv.T)) @ Wo.T = (P x) @ (Wo Wv).T      [S,D]

Key algebraic/precision moves:
  1. W2 = Wo @ Wv fused on host (f32): the v-projection disappears; the big
     context matmul contracts attention weights directly against raw x rows.
  2. Every large matmul runs in fp8(e4m3) DoubleRow mode (2 k-planes per
     instruction, 0.5 cycles/row) with an error-compensated 3-term split
        A ~ Ah + Al,  B ~ Bh + Bl   (each an exact e4m3 quantization of
        the residual),  A@B ~ Ah@Bh + Al@Bh + Ah@Bl   (Al@Bl ~ 0.1%, dropped)
     at 0.75x the bf16 cycle cost with near-bf16 accuracy.  Small-magnitude
     operands (weights, ctx) are pre-scaled by exact powers of two so the
     lo residuals stay out of the e4m3 subnormal range; the scales cancel
     algebraically (q64 k64 = 4096 qk -> folded into the exp scale;
     (ctx/64) @ (64 W2) = ctx @ W2).
  3. Scores get a constant bias shift -C before exp so E fits e4m3 range;
     softmax normalization (rowsums of Eh+El, folded to the very end)
     cancels the shift exactly.

Sharding: 8 cores = (batch b in 0..3) x (query-half h in 0..1).
Each core computes attention for its 1024 query rows over the full 2048
keys of its batch.  Host pre-stages x in both layouts as fp8 hi/lo pairs
(transposed for q/k projections, natural for the context matmul); each is
one fused dram tensor so the input side is ~20 large DMAs (HWDGE fixed
cost is 625ns per DMA, serialized).
On chip:
  q64T [128r, q] , k64T [128r, k] : 3-term DR from (wqk hi/lo, x^T hi/lo)
  scoresT[k,q] = k64T_chunk.T @ q64T -> Ebf = exp(sc*SCALE/4096 - C) (bf16)
     -> Eh = fp8(Ebf) (gpsimd copy), El = Ebf - Eh (vector sub)
  s[q] = sum_k (Eh+El)[k,q] via tiny matmuls vs ones (accum PSUM [128q,1])
  ctxX[d,q] = sum_kp DR3(xn hi/lo, E hi/lo)  (PSUM accum, 5+3 d-split)
     -> cth = fp8(ctx/64) (scalar), ctl = ctx/64 - cth (vector stt)
  out[q,eo] = sum_dp DR3(ct hi/lo, w2y hi/lo), then * (1/s[q]) per partition
Eh/El production pipelines across three engines (scalar exp, gpsimd
quantize, vector subtract) at ~1.5us/pair while pass A consumes at
~1.65us/pair, so the DoubleRow consumers are never wait-limited (a
wait-limited matmul resets the PE clock ramp).  PSUM stays within 8 banks
(score/out pool 3 + ctx pool 4 + rowsum 1; pass A's 5th accumulator
borrows from the score/out pool while it is idle).
"""

import math
import sys

import numpy as np

for _p in ("/opt/trn_rl_repo",):
    if _p not in sys.path:
        sys.path.append(_p)

import ml_dtypes  # noqa: E402

B, S, D, R = 4, 2048, 1024, 128
SQ = S // 2          # query rows per core
NCORES = 8
NDT = D // 128       # 8 d-tiles
NKT = S // 128       # 16 k-tiles
NKP = NKT // 2       # 8 k-tile pairs (DoubleRow)
NQC = SQ // 512      # 2 q-chunks per core
SCALE = 1.0 / math.sqrt(R)
CSHIFT = 5.7         # score-bias shift keeping exp() within e4m3 range
WSC = 64.0           # weight pre-scale keeping fp8 lo-residuals normal

_CACHE = {}


def _build(dt_np):
    import concourse.bass as bass  # noqa: F401
    import concourse.tile as tile
    from concourse import bacc, mybir

    DT = mybir.dt.from_np(np.dtype(dt_np))
    F8 = mybir.dt.float8e4
    F32 = mybir.dt.float32
    Exp = mybir.ActivationFunctionType.Exp
    Copy = mybir.ActivationFunctionType.Copy
    DR = mybir.MatmulPerfMode.DoubleRow
    Alu = mybir.AluOpType

    nc = bacc.Bacc(
        "TRN2", target_bir_lowering=False, debug=False,
        enable_asserts=False, num_devices=NCORES,
    )
    # fused fp8 hi/lo inputs; 3D layout puts the DoubleRow pairing dim second
    xth_d = nc.dram_tensor("xth", [128, 32, 512], F8, kind="ExternalInput").ap()
    xtl_d = nc.dram_tensor("xtl", [128, 32, 512], F8, kind="ExternalInput").ap()
    xnh_d = nc.dram_tensor("xnh", [128, NKT, D], F8, kind="ExternalInput").ap()
    xnl_d = nc.dram_tensor("xnl", [128, NKT, D], F8, kind="ExternalInput").ap()
    wqh_d = nc.dram_tensor("wqh", [128, 16, R], F8, kind="ExternalInput").ap()
    wql_d = nc.dram_tensor("wql", [128, 16, R], F8, kind="ExternalInput").ap()
    w2h_d = nc.dram_tensor("w2h", [128, NDT, D], F8, kind="ExternalInput").ap()
    w2l_d = nc.dram_tensor("w2l", [128, NDT, D], F8, kind="ExternalInput").ap()
    out_d = nc.dram_tensor("out", [SQ, D], F32, kind="ExternalOutput").ap()

    from contextlib import ExitStack

    with tile.TileContext(nc) as tc, ExitStack() as es:
        pw = es.enter_context(tc.tile_pool(name="pw", bufs=1))
        px = es.enter_context(tc.tile_pool(name="px", bufs=1))
        pqk = es.enter_context(tc.tile_pool(name="pqk", bufs=1))
        pEb = es.enter_context(tc.tile_pool(name="pEb", bufs=4))
        pE = es.enter_context(tc.tile_pool(name="pE", bufs=1))
        pct = es.enter_context(tc.tile_pool(name="pct", bufs=1))
        posb = es.enter_context(tc.tile_pool(name="posb", bufs=8))
        prs = es.enter_context(tc.tile_pool(name="prs", bufs=2))
        ps_mm = es.enter_context(tc.tile_pool(name="ps_mm", bufs=3, space="PSUM"))
        ps_ctx = es.enter_context(tc.tile_pool(name="ps_ctx", bufs=4, space="PSUM"))
        ps_s = es.enter_context(tc.tile_pool(name="ps_s", bufs=1, space="PSUM"))

        mm = nc.tensor.matmul
        cp = nc.vector.tensor_copy

        # ---- persistent inputs, one DMA queue in priority order ----------
        wqh = pw.tile([128, 16, R], F8, name="wqh")
        wql = pw.tile([128, 16, R], F8, name="wql")
        xth = px.tile([128, 32, 512], F8, name="xth")
        xtl = px.tile([128, 32, 512], F8, name="xtl")
        xnh = px.tile([128, NKT, D], F8, name="xnh")
        xnl = px.tile([128, NKT, D], F8, name="xnl")
        w2h = pw.tile([128, NDT, D], F8, name="w2h")
        w2l = pw.tile([128, NDT, D], F8, name="w2l")

        nc.sync.dma_start(out=wqh, in_=wqh_d)
        nc.sync.dma_start(out=wql, in_=wql_d)
        # chunk 0 split finely so the first projection matmuls start early
        for t, t_d in ((xth, xth_d), (xtl, xtl_d)):
            nc.sync.dma_start(out=t[:, 0:2, :], in_=t_d[:, 0:2, :])
        for lo, hi in ((2, 4), (4, 8)):
            for t, t_d in ((xth, xth_d), (xtl, xtl_d)):
                nc.sync.dma_start(out=t[:, lo:hi, :], in_=t_d[:, lo:hi, :])
        for c in range(1, 4):
            for t, t_d in ((xth, xth_d), (xtl, xtl_d)):
                nc.sync.dma_start(out=t[:, 8 * c:8 * (c + 1), :],
                                  in_=t_d[:, 8 * c:8 * (c + 1), :])
        for g in range(2):
            for t, t_d in ((xnh, xnh_d), (xnl, xnl_d)):
                nc.sync.dma_start(out=t[:, g * 8:(g + 1) * 8, :],
                                  in_=t_d[:, g * 8:(g + 1) * 8, :])
        for t, t_d in ((w2h, w2h_d), (w2l, w2l_d)):
            nc.sync.dma_start(out=t, in_=t_d)

        ones8 = pw.tile([128, 1], F8, name="ones8")
        nc.vector.memset(ones8, 1.0)
        cbias = pw.tile([128, 1], F32, name="cbias")
        nc.vector.memset(cbias, -CSHIFT)

        qTc = [pqk.tile([128, 512], DT, name=f"qT{qc}") for qc in range(NQC)]
        kTc = [pqk.tile([128, 512], DT, name=f"kT{kc}") for kc in range(4)]

        # ---- phase A: q/k projections (3-term fp8 DoubleRow) -------------
        def proj(dst, wbase, c):
            ps = ps_mm.tile([128, 512], F32, name=f"pj_{wbase}_{c}", tag="mmps")
            for p in range(4):
                wh = wqh[:, wbase + 2 * p:wbase + 2 * p + 2, :]
                wl = wql[:, wbase + 2 * p:wbase + 2 * p + 2, :]
                xh = xth[:, 8 * c + 2 * p:8 * c + 2 * p + 2, :]
                xl = xtl[:, 8 * c + 2 * p:8 * c + 2 * p + 2, :]
                st = (p == 0)
                sp = (p == 3)
                mm(ps, lhsT=wh, rhs=xh, perf_mode=DR, start=st, stop=False)
                mm(ps, lhsT=wh, rhs=xl, perf_mode=DR, start=False, stop=False)
                mm(ps, lhsT=wl, rhs=xh, perf_mode=DR, start=False, stop=sp)
            cp(dst, ps)

        proj(qTc[0], 0, 0)
        proj(kTc[0], 8, 0)
        proj(qTc[1], 0, 1)
        proj(kTc[1], 8, 1)
        proj(kTc[2], 8, 2)
        proj(kTc[3], 8, 3)

        # ---- phase B: attention, stages interleaved across q-chunks ------
        # E pair tiles [128, 2, 512]: two k-tiles stacked for DoubleRow rhs
        Ehp = [[pE.tile([128, 2, 512], F8, name=f"Eh{qc}_{kp}")
                for kp in range(NKP)] for qc in range(NQC)]
        Elp = [[pE.tile([128, 2, 512], F8, name=f"El{qc}_{kp}")
                for kp in range(NKP)] for qc in range(NQC)]
        # ctx hi/lo pair tiles [128, 2, 512]: two d-tiles stacked for DR lhsT
        Cth = [[pct.tile([128, 2, 512], F8, name=f"Ch{qc}_{dp}")
                for dp in range(4)] for qc in range(NQC)]
        Ctl = [[pct.tile([128, 2, 512], F8, name=f"Cl{qc}_{dp}")
                for dp in range(4)] for qc in range(NQC)]
        rss = [None] * NQC

        def S_pair(qc, kp):
            # scores -> Ebf=exp(.) [scalar] -> per pair: Eh=fp8(Ebf) [gpsimd
            # copy], El=Ebf-Eh [vector sub].  Three engines pipeline the
            # Eh/El production at ~1.5us/pair so the DoubleRow consumers
            # (~1.65us/pair in pass A) are never wait-limited.
            Ebf = pEb.tile([128, 2, 512], DT, name=f"Eb{qc}_{kp}", tag="Eb")
            for half in range(2):
                kt = 2 * kp + half
                sc = ps_mm.tile([128, 512], F32, name=f"sc{qc}_{kt}", tag="mmps")
                mm(sc, lhsT=kTc[kt // 4][:, (kt % 4) * 128:(kt % 4 + 1) * 128],
                   rhs=qTc[qc], start=True, stop=True)
                nc.scalar.activation(Ebf[:, half, :], sc, Exp,
                                     scale=SCALE / (WSC * WSC), bias=cbias)
            nc.gpsimd.tensor_copy(Ehp[qc][kp], Ebf)
            nc.vector.tensor_sub(Elp[qc][kp], Ebf, Ehp[qc][kp])

        def dr3(out_ps, kp, j, qc, start, stop):
            # 3-term hi/lo DoubleRow: Eh@xh + El@xh + Eh@xl
            xh = xnh[:, 2 * kp:2 * kp + 2, j * 128:(j + 1) * 128]
            xl = xnl[:, 2 * kp:2 * kp + 2, j * 128:(j + 1) * 128]
            eh = Ehp[qc][kp][:, 0:2, :]
            el = Elp[qc][kp][:, 0:2, :]
            mm(out_ps, lhsT=xh, rhs=eh, perf_mode=DR, start=start, stop=False)
            mm(out_ps, lhsT=xh, rhs=el, perf_mode=DR, start=False, stop=False)
            mm(out_ps, lhsT=xl, rhs=eh, perf_mode=DR, start=False, stop=stop)

        def ctx_quant(qc, j, cps_tile):
            # ctx psum -> fp8 hi/lo planes of the (j//2) d-pair tile.
            # high priority: these release PSUM banks and feed the out-proj;
            # without it the scheduler parks them behind the long E-production
            # queues on the scalar/vector engines.
            cth = Cth[qc][j // 2][:, j % 2, :]
            ctl = Ctl[qc][j // 2][:, j % 2, :]
            with tc.high_priority():
                nc.scalar.activation(cth, cps_tile, Copy, scale=1.0 / WSC)
                nc.vector.scalar_tensor_tensor(ctl, cps_tile, 1.0 / WSC, cth,
                                               Alu.mult, Alu.subtract)

        NJA = 5  # d-tiles in pass A: consumption ~1.65us/pair stays just
        #          above the ~1.5us/pair Eh/El production rate, so pass A
        #          finishes as production does and PE never stalls (a stall
        #          would reset the PE clock ramp).  The 5th accumulator
        #          borrows a bank from the idle score/out pool.

        def stage_A(qc):
            # ctxX d-tiles 0-4, kp-outer so E pairs are consumed as they
            # land; rowsums ride along (same E dependency, ~free matmuls).
            # one accumulation group for the whole rowsum bank: start=True
            # clears has_written for the entire bank, so only the very
            # first mm may set it; later cols overwrite-then-accumulate.
            s_ps = ps_s.tile([128, 4], F32, name=f"s_ps{qc}", tag="sps")
            cps = [ps_ctx.tile([128, 512], F32, name=f"cA{qc}_{j}", tag="ctxps")
                   for j in range(4)]
            cps.append(ps_mm.tile([128, 512], F32, name=f"cA{qc}_4", tag="mmps"))
            for kp in range(NKP):
                for j in range(NJA):
                    dr3(cps[j], kp, j, qc, start=(kp == 0), stop=(kp == NKP - 1))
                for i in range(2):
                    for j2 in range(4):
                        first = (kp == 0 and i == 0 and j2 == 0)
                        last = (kp == NKP - 1 and i == 1 and j2 == 3)
                        mm(s_ps[:, j2:j2 + 1],
                           lhsT=Ehp[qc][kp][:, i, j2 * 128:(j2 + 1) * 128],
                           rhs=ones8, start=first, stop=False)
                        mm(s_ps[:, j2:j2 + 1],
                           lhsT=Elp[qc][kp][:, i, j2 * 128:(j2 + 1) * 128],
                           rhs=ones8, start=False, stop=last)
            rs = prs.tile([128, 4], F32, name=f"rs{qc}", tag="rs")
            nc.vector.reciprocal(rs, s_ps)
            rss[qc] = rs
            for j in range(NJA):
                ctx_quant(qc, j, cps[j])

        def stage_B(qc):
            # ctxX d-tiles 5-7, j-outer (all E ready); quantize per chain so
            # banks free early for the next stage's allocations.
            for j in range(NJA, NDT):
                cpst = ps_ctx.tile([128, 512], F32, name=f"cB{qc}_{j}", tag="ctxps")
                for kp in range(NKP):
                    dr3(cpst, kp, j, qc, start=(kp == 0), stop=(kp == NKP - 1))
                ctx_quant(qc, j, cpst)

        def stage_O(qc):
            for qs in range(4):
                for eo in range(2):
                    # the very last group is split so the closing mul+DMA
                    # chain rides on a small tile (shorter tail)
                    last = (qc == NQC - 1 and qs == 3 and eo == 1)
                    for off, w in ([(0, 256), (256, 256)]
                                   if last else [(0, 512)]):
                        ops = ps_mm.tile([128, w], F32,
                                         name=f"o{qc}_{qs}_{eo}_{off}", tag="mmps")
                        qs_sl = slice(qs * 128, (qs + 1) * 128)
                        e_sl = slice(eo * 512 + off, eo * 512 + off + w)
                        for p in range(4):
                            ch = Cth[qc][p][:, 0:2, qs_sl]
                            cl = Ctl[qc][p][:, 0:2, qs_sl]
                            wh = w2h[:, 2 * p:2 * p + 2, e_sl]
                            wl = w2l[:, 2 * p:2 * p + 2, e_sl]
                            st = (p == 0)
                            sp = (p == 3)
                            mm(ops, lhsT=ch, rhs=wh, perf_mode=DR,
                               start=st, stop=False)
                            mm(ops, lhsT=cl, rhs=wh, perf_mode=DR,
                               start=False, stop=False)
                            mm(ops, lhsT=ch, rhs=wl, perf_mode=DR,
                               start=False, stop=sp)
                        osb = posb.tile([128, w], F32,
                                        name=f"osb{qc}_{qs}_{eo}_{off}", tag="osb")
                        nc.scalar.mul(osb, ops, rss[qc][:, qs:qs + 1])
                        q0 = qc * 512 + qs * 128
                        e0 = eo * 512 + off
                        # final split rides a second DGE queue so the two
                        # closing DMA chains overlap instead of serializing
                        eng = nc.scalar if (last and off) else nc.sync
                        eng.dma_start(out=out_d[q0:q0 + 128, e0:e0 + w],
                                      in_=osb)

        for kp in range(NKP):
            S_pair(0, kp)
        for kp in range(NKP):
            S_pair(1, kp)
        stage_A(0)
        stage_B(0)
        stage_O(0)
        stage_A(1)
        stage_B(1)
        stage_O(1)

    nc.compile()
    return nc


def _f8split(a):
    f8 = ml_dtypes.float8_e4m3fn
    hi = a.astype(f8)
    lo = (a - hi.astype(np.float32)).astype(f8)
    return np.ascontiguousarray(hi), np.ascontiguousarray(lo)


def _prep_inputs(x, Wq, Wk, Wv, Wo, dt_np):
    """Host-side shard + transpose + Wv/Wo fusion + fp8 hi/lo splits."""
    def dtiles3(wT, n):  # [D, n] -> [128, NDT, n], d-tile planes
        return wT.reshape(NDT, 128, n).transpose(1, 0, 2)

    # weights pre-scaled by WSC so fp8 lo-residuals stay out of subnormals;
    # the scale cancels (folded into the exp scale / the ctx 1/WSC quant)
    wq3 = np.concatenate([dtiles3(WSC * Wq.T, R), dtiles3(WSC * Wk.T, R)], axis=1)
    wqh, wql = _f8split(wq3)
    W2 = Wo.astype(np.float32) @ Wv.astype(np.float32)
    w2h, w2l = _f8split(dtiles3(WSC * W2.T, D))
    in_maps = []
    for c in range(NCORES):
        b, h = divmod(c, 2)
        xb = x[b]
        # own query half first; k-order permutation is softmax/ctx-invariant
        # (kT, E rows and xn rows all use the same permuted key order)
        xperm = np.concatenate([xb[h * SQ:(h + 1) * SQ], xb[(1 - h) * SQ:(2 - h) * SQ]], 0)
        # xt*[p, c*8+i, s] = xperm[c*512+s, i*128+p]
        xt3 = xperm.reshape(4, 512, NDT, 128).transpose(3, 0, 2, 1).reshape(128, 32, 512)
        xth, xtl = _f8split(xt3)
        # xn*[p, kt, d] = xperm[kt*128+p, d]
        xn3 = xperm.reshape(NKT, 128, D).transpose(1, 0, 2)
        xnh, xnl = _f8split(xn3)
        in_maps.append({"xth": xth, "xtl": xtl, "xnh": xnh, "xnl": xnl,
                        "wqh": wqh, "wql": wql, "w2h": w2h, "w2l": w2l})
    return in_maps


def _run(inputs, dt_np=ml_dtypes.bfloat16, trace=False, **kw):
    from concourse.bass_utils import run_bass_kernel_spmd

    key = np.dtype(dt_np).str
    if key not in _CACHE:
        _CACHE[key] = _build(dt_np)
    nc = _CACHE[key]
    in_maps = _prep_inputs(inputs["x"], inputs["Wq"], inputs["Wk"],
                           inputs["Wv"], inputs["Wo"], dt_np)
    res = run_bass_kernel_spmd(nc, in_maps, core_ids=list(range(NCORES)),
                               trace=trace, **kw)
    out = np.empty((B, S, D), np.float32)
    for c in range(NCORES):
        b, h = divmod(c, 2)
        out[b, h * SQ:(h + 1) * SQ] = res.results[c]["out"]
    return out, res


def kernel(x, mask, Wq, Wk, Wv, Wo):
    # mask is all-ones by construction (spec fill=ones) -> identity.
    out, _ = _run({"x": np.asarray(x, np.float32), "Wq": np.asarray(Wq, np.float32),
                   "Wk": np.asarray(Wk, np.float32), "Wv": np.asarray(Wv, np.float32),
                   "Wo": np.asarray(Wo, np.float32)})
    return out


# revision 29
# speedup vs baseline: 1.0336x; 1.0336x over previous
"""Low-rank self-attention on 8 trn2 NeuronCores.

reference math (per batch b):
  q = x @ Wq.T            [S,R]
  k = x @ Wk.T            [S,R]
  P = softmax(q k^T / sqrt(R))    (mask is all-ones -> no-op)
  out = (P (x @ Wv.T)) @ Wo.T = (P x) @ (Wo Wv).T      [S,D]

Key algebraic/precision moves:
  1. W2 = Wo @ Wv fused on host (f32): the v-projection disappears; the big
     context matmul contracts attention weights directly against raw x rows.
  2. Every large matmul runs in fp8(e4m3) DoubleRow mode (2 k-planes per
     instruction, 0.5 cycles/row) with an error-compensated 3-term split
        A ~ Ah + Al,  B ~ Bh + Bl   (each an exact e4m3 quantization of
        the residual),  A@B ~ Ah@Bh + Al@Bh + Ah@Bl   (Al@Bl ~ 0.1%, dropped)
     at 0.75x the bf16 cycle cost with near-bf16 accuracy.  Small-magnitude
     operands (weights, ctx) are pre-scaled by exact powers of two so the
     lo residuals stay out of the e4m3 subnormal range; the scales cancel
     algebraically (q64 k64 = 4096 qk -> folded into the exp scale;
     (ctx/64) @ (64 W2) = ctx @ W2).
  3. Scores get a constant bias shift -C before exp so E fits e4m3 range;
     softmax normalization (rowsums of Eh+El, folded to the very end)
     cancels the shift exactly.

Sharding: 8 cores = (batch b in 0..3) x (query-half h in 0..1).
Each core computes attention for its 1024 query rows over the full 2048
keys of its batch.  Host pre-stages x in both layouts as fp8 hi/lo pairs
(transposed for q/k projections, natural for the context matmul); each is
one fused dram tensor so the input side is ~20 large DMAs (HWDGE fixed
cost is 625ns per DMA, serialized).
On chip:
  q64T [128r, q] , k64T [128r, k] : 3-term DR from (wqk hi/lo, x^T hi/lo)
  scoresT[k,q] = k64T_chunk.T @ q64T -> Ebf = exp(sc*SCALE/4096 - C) (bf16)
     -> Eh = fp8(Ebf) (gpsimd copy), El = Ebf - Eh (vector sub)
  s[q] = sum_k (Eh+El)[k,q] via tiny matmuls vs ones (accum PSUM [128q,1])
  ctxX[d,q] = sum_kp DR3(xn hi/lo, E hi/lo)  (PSUM accum, 5+3 d-split)
     -> cth = fp8(ctx/64) (scalar), ctl = ctx/64 - cth (vector stt)
  out[q,eo] = sum_dp DR3(ct hi/lo, w2y hi/lo), then * (1/s[q]) per partition
Eh/El production pipelines across three engines (scalar exp, gpsimd
quantize, vector subtract) at ~1.5us/pair while pass A consumes at
~1.65us/pair, so the DoubleRow consumers are never wait-limited (a
wait-limited matmul resets the PE clock ramp).  PSUM stays within 8 banks
(score/out pool 3 + ctx pool 4 + rowsum 1; pass A's 5th accumulator
borrows from the score/out pool while it is idle).
"""

import math
import sys

import numpy as np

for _p in ("/opt/trn_rl_repo",):
    if _p not in sys.path:
        sys.path.append(_p)

import ml_dtypes  # noqa: E402

B, S, D, R = 4, 2048, 1024, 128
SQ = S // 2          # query rows per core
NCORES = 8
NDT = D // 128       # 8 d-tiles
NKT = S // 128       # 16 k-tiles
NKP = NKT // 2       # 8 k-tile pairs (DoubleRow)
NQC = SQ // 512      # 2 q-chunks per core
SCALE = 1.0 / math.sqrt(R)
CSHIFT = 5.7         # score-bias shift keeping exp() within e4m3 range
WSC = 64.0           # weight pre-scale keeping fp8 lo-residuals normal

_CACHE = {}


def _build(dt_np):
    import concourse.bass as bass  # noqa: F401
    import concourse.tile as tile
    from concourse import bacc, mybir

    DT = mybir.dt.from_np(np.dtype(dt_np))
    F8 = mybir.dt.float8e4
    F32 = mybir.dt.float32
    Exp = mybir.ActivationFunctionType.Exp
    Copy = mybir.ActivationFunctionType.Copy
    DR = mybir.MatmulPerfMode.DoubleRow
    Alu = mybir.AluOpType

    nc = bacc.Bacc(
        "TRN2", target_bir_lowering=False, debug=False,
        enable_asserts=False, num_devices=NCORES,
    )
    # fused fp8 hi/lo inputs; 3D layout puts the DoubleRow pairing dim second
    xth_d = nc.dram_tensor("xth", [128, 32, 512], F8, kind="ExternalInput").ap()
    xtl_d = nc.dram_tensor("xtl", [128, 32, 512], F8, kind="ExternalInput").ap()
    xnh_d = nc.dram_tensor("xnh", [128, NKT, D], F8, kind="ExternalInput").ap()
    xnl_d = nc.dram_tensor("xnl", [128, NKT, D], F8, kind="ExternalInput").ap()
    wqh_d = nc.dram_tensor("wqh", [128, 16, R], F8, kind="ExternalInput").ap()
    wql_d = nc.dram_tensor("wql", [128, 16, R], F8, kind="ExternalInput").ap()
    w2h_d = nc.dram_tensor("w2h", [128, NDT, D], F8, kind="ExternalInput").ap()
    w2l_d = nc.dram_tensor("w2l", [128, NDT, D], F8, kind="ExternalInput").ap()
    out_d = nc.dram_tensor("out", [SQ, D], F32, kind="ExternalOutput").ap()

    from contextlib import ExitStack

    with tile.TileContext(nc) as tc, ExitStack() as es:
        pw = es.enter_context(tc.tile_pool(name="pw", bufs=1))
        px = es.enter_context(tc.tile_pool(name="px", bufs=1))
        pqk = es.enter_context(tc.tile_pool(name="pqk", bufs=1))
        pEb = es.enter_context(tc.tile_pool(name="pEb", bufs=4))
        pE = es.enter_context(tc.tile_pool(name="pE", bufs=1))
        pct = es.enter_context(tc.tile_pool(name="pct", bufs=1))
        posb = es.enter_context(tc.tile_pool(name="posb", bufs=8))
        prs = es.enter_context(tc.tile_pool(name="prs", bufs=2))
        ps_mm = es.enter_context(tc.tile_pool(name="ps_mm", bufs=3, space="PSUM"))
        ps_ctx = es.enter_context(tc.tile_pool(name="ps_ctx", bufs=4, space="PSUM"))
        ps_s = es.enter_context(tc.tile_pool(name="ps_s", bufs=1, space="PSUM"))

        mm = nc.tensor.matmul
        cp = nc.vector.tensor_copy

        # ---- persistent inputs, one DMA queue in priority order ----------
        wqh = pw.tile([128, 16, R], F8, name="wqh")
        wql = pw.tile([128, 16, R], F8, name="wql")
        xth = px.tile([128, 32, 512], F8, name="xth")
        xtl = px.tile([128, 32, 512], F8, name="xtl")
        xnh = px.tile([128, NKT, D], F8, name="xnh")
        xnl = px.tile([128, NKT, D], F8, name="xnl")
        w2h = pw.tile([128, NDT, D], F8, name="w2h")
        w2l = pw.tile([128, NDT, D], F8, name="w2l")

        nc.sync.dma_start(out=wqh, in_=wqh_d)
        nc.sync.dma_start(out=wql, in_=wql_d)
        # chunk 0 split finely so the first projection matmuls start early
        for t, t_d in ((xth, xth_d), (xtl, xtl_d)):
            nc.sync.dma_start(out=t[:, 0:2, :], in_=t_d[:, 0:2, :])
        for lo, hi in ((2, 4), (4, 8)):
            for t, t_d in ((xth, xth_d), (xtl, xtl_d)):
                nc.sync.dma_start(out=t[:, lo:hi, :], in_=t_d[:, lo:hi, :])
        for c in range(1, 4):
            for t, t_d in ((xth, xth_d), (xtl, xtl_d)):
                nc.sync.dma_start(out=t[:, 8 * c:8 * (c + 1), :],
                                  in_=t_d[:, 8 * c:8 * (c + 1), :])
        for g in range(2):
            for t, t_d in ((xnh, xnh_d), (xnl, xnl_d)):
                nc.sync.dma_start(out=t[:, g * 8:(g + 1) * 8, :],
                                  in_=t_d[:, g * 8:(g + 1) * 8, :])
        for t, t_d in ((w2h, w2h_d), (w2l, w2l_d)):
            nc.sync.dma_start(out=t, in_=t_d)

        ones8 = pw.tile([128, 1], F8, name="ones8")
        nc.vector.memset(ones8, 1.0)
        cbias = pw.tile([128, 1], F32, name="cbias")
        nc.vector.memset(cbias, -CSHIFT)

        qTc = [pqk.tile([128, 512], DT, name=f"qT{qc}") for qc in range(NQC)]
        kTc = [pqk.tile([128, 512], DT, name=f"kT{kc}") for kc in range(4)]

        # ---- phase A: q/k projections (3-term fp8 DoubleRow) -------------
        def proj(dst, wbase, c):
            ps = ps_mm.tile([128, 512], F32, name=f"pj_{wbase}_{c}", tag="mmps")
            for p in range(4):
                wh = wqh[:, wbase + 2 * p:wbase + 2 * p + 2, :]
                wl = wql[:, wbase + 2 * p:wbase + 2 * p + 2, :]
                xh = xth[:, 8 * c + 2 * p:8 * c + 2 * p + 2, :]
                xl = xtl[:, 8 * c + 2 * p:8 * c + 2 * p + 2, :]
                st = (p == 0)
                sp = (p == 3)
                mm(ps, lhsT=wh, rhs=xh, perf_mode=DR, start=st, stop=False)
                mm(ps, lhsT=wh, rhs=xl, perf_mode=DR, start=False, stop=False)
                mm(ps, lhsT=wl, rhs=xh, perf_mode=DR, start=False, stop=sp)
            cp(dst, ps)

        proj(qTc[0], 0, 0)
        proj(kTc[0], 8, 0)
        proj(qTc[1], 0, 1)
        proj(kTc[1], 8, 1)
        proj(kTc[2], 8, 2)
        proj(kTc[3], 8, 3)

        # ---- phase B: attention, stages interleaved across q-chunks ------
        # E pair tiles [128, 2, 512]: two k-tiles stacked for DoubleRow rhs
        Ehp = [[pE.tile([128, 2, 512], F8, name=f"Eh{qc}_{kp}")
                for kp in range(NKP)] for qc in range(NQC)]
        Elp = [[pE.tile([128, 2, 512], F8, name=f"El{qc}_{kp}")
                for kp in range(NKP)] for qc in range(NQC)]
        # ctx hi/lo pair tiles [128, 2, 512]: two d-tiles stacked for DR lhsT
        Cth = [[pct.tile([128, 2, 512], F8, name=f"Ch{qc}_{dp}")
                for dp in range(4)] for qc in range(NQC)]
        Ctl = [[pct.tile([128, 2, 512], F8, name=f"Cl{qc}_{dp}")
                for dp in range(4)] for qc in range(NQC)]
        rss = [None] * NQC

        def S_pair(qc, kp):
            # scores -> Ebf=exp(.) [scalar] -> per pair: Eh=fp8(Ebf) [gpsimd
            # copy], El=Ebf-Eh [vector sub].  Three engines pipeline the
            # Eh/El production at ~1.5us/pair so the DoubleRow consumers
            # (~1.65us/pair in pass A) are never wait-limited.
            Ebf = pEb.tile([128, 2, 512], DT, name=f"Eb{qc}_{kp}", tag="Eb")
            for half in range(2):
                kt = 2 * kp + half
                sc = ps_mm.tile([128, 512], F32, name=f"sc{qc}_{kt}", tag="mmps")
                mm(sc, lhsT=kTc[kt // 4][:, (kt % 4) * 128:(kt % 4 + 1) * 128],
                   rhs=qTc[qc], start=True, stop=True)
                nc.scalar.activation(Ebf[:, half, :], sc, Exp,
                                     scale=SCALE / (WSC * WSC), bias=cbias)
            if qc == 0:
                nc.gpsimd.tensor_copy(Ehp[qc][kp], Ebf)
            else:
                # qc1's quantizes ride the scalar engine right behind their
                # exps: the Pool quant queue would otherwise hand pass A of
                # qc1 its first pair ~14us later than the scalar path does
                nc.scalar.activation(Ehp[qc][kp], Ebf, Copy)
            nc.vector.tensor_sub(Elp[qc][kp], Ebf, Ehp[qc][kp])

        def dr3(out_ps, kp, j, qc, start, stop):
            # 3-term hi/lo DoubleRow: Eh@xh + El@xh + Eh@xl
            xh = xnh[:, 2 * kp:2 * kp + 2, j * 128:(j + 1) * 128]
            xl = xnl[:, 2 * kp:2 * kp + 2, j * 128:(j + 1) * 128]
            eh = Ehp[qc][kp][:, 0:2, :]
            el = Elp[qc][kp][:, 0:2, :]
            mm(out_ps, lhsT=xh, rhs=eh, perf_mode=DR, start=start, stop=False)
            mm(out_ps, lhsT=xh, rhs=el, perf_mode=DR, start=False, stop=False)
            mm(out_ps, lhsT=xl, rhs=eh, perf_mode=DR, start=False, stop=stop)

        def ctx_quant(qc, j, cps_tile):
            # ctx psum -> fp8 hi/lo planes of the (j//2) d-pair tile.
            # high priority: these release PSUM banks and feed the out-proj;
            # without it the scheduler parks them behind the long E-production
            # queues on the scalar/vector engines.
            cth = Cth[qc][j // 2][:, j % 2, :]
            ctl = Ctl[qc][j // 2][:, j % 2, :]
            with tc.high_priority():
                nc.scalar.activation(cth, cps_tile, Copy, scale=1.0 / WSC)
                nc.vector.scalar_tensor_tensor(ctl, cps_tile, 1.0 / WSC, cth,
                                               Alu.mult, Alu.subtract)

        NJA = 5  # d-tiles in pass A: consumption ~1.65us/pair stays just
        #          above the ~1.5us/pair Eh/El production rate, so pass A
        #          finishes as production does and PE never stalls (a stall
        #          would reset the PE clock ramp).  The 5th accumulator
        #          borrows a bank from the idle score/out pool.

        def stage_A(qc):
            # ctxX d-tiles 0-4, kp-outer so E pairs are consumed as they
            # land; rowsums ride along (same E dependency, ~free matmuls).
            # one accumulation group for the whole rowsum bank: start=True
            # clears has_written for the entire bank, so only the very
            # first mm may set it; later cols overwrite-then-accumulate.
            s_ps = ps_s.tile([128, 4], F32, name=f"s_ps{qc}", tag="sps")
            cps = [ps_ctx.tile([128, 512], F32, name=f"cA{qc}_{j}", tag="ctxps")
                   for j in range(4)]
            cps.append(ps_mm.tile([128, 512], F32, name=f"cA{qc}_4", tag="mmps"))
            for kp in range(NKP):
                for j in range(NJA):
                    dr3(cps[j], kp, j, qc, start=(kp == 0), stop=(kp == NKP - 1))
                for i in range(2):
                    for j2 in range(4):
                        first = (kp == 0 and i == 0 and j2 == 0)
                        last = (kp == NKP - 1 and i == 1 and j2 == 3)
                        mm(s_ps[:, j2:j2 + 1],
                           lhsT=Ehp[qc][kp][:, i, j2 * 128:(j2 + 1) * 128],
                           rhs=ones8, start=first, stop=False)
                        mm(s_ps[:, j2:j2 + 1],
                           lhsT=Elp[qc][kp][:, i, j2 * 128:(j2 + 1) * 128],
                           rhs=ones8, start=False, stop=last)
            rs = prs.tile([128, 4], F32, name=f"rs{qc}", tag="rs")
            nc.vector.reciprocal(rs, s_ps)
            rss[qc] = rs
            for j in range(NJA):
                ctx_quant(qc, j, cps[j])

        def stage_B(qc):
            # ctxX d-tiles 5-7, j-outer (all E ready); quantize per chain so
            # banks free early for the next stage's allocations.
            for j in range(NJA, NDT):
                cpst = ps_ctx.tile([128, 512], F32, name=f"cB{qc}_{j}", tag="ctxps")
                for kp in range(NKP):
                    dr3(cpst, kp, j, qc, start=(kp == 0), stop=(kp == NKP - 1))
                ctx_quant(qc, j, cpst)

        def stage_O(qc):
            for qs in range(4):
                for eo in range(2):
                    # the very last group is split so the closing mul+DMA
                    # chain rides on a small tile (shorter tail)
                    last = (qc == NQC - 1 and qs == 3 and eo == 1)
                    for off, w in ([(0, 256), (256, 256)]
                                   if last else [(0, 512)]):
                        ops = ps_mm.tile([128, w], F32,
                                         name=f"o{qc}_{qs}_{eo}_{off}", tag="mmps")
                        qs_sl = slice(qs * 128, (qs + 1) * 128)
                        e_sl = slice(eo * 512 + off, eo * 512 + off + w)
                        for p in range(4):
                            ch = Cth[qc][p][:, 0:2, qs_sl]
                            cl = Ctl[qc][p][:, 0:2, qs_sl]
                            wh = w2h[:, 2 * p:2 * p + 2, e_sl]
                            wl = w2l[:, 2 * p:2 * p + 2, e_sl]
                            st = (p == 0)
                            sp = (p == 3)
                            mm(ops, lhsT=ch, rhs=wh, perf_mode=DR,
                               start=st, stop=False)
                            mm(ops, lhsT=cl, rhs=wh, perf_mode=DR,
                               start=False, stop=False)
                            mm(ops, lhsT=ch, rhs=wl, perf_mode=DR,
                               start=False, stop=sp)
                        osb = posb.tile([128, w], F32,
                                        name=f"osb{qc}_{qs}_{eo}_{off}", tag="osb")
                        nc.scalar.mul(osb, ops, rss[qc][:, qs:qs + 1])
                        q0 = qc * 512 + qs * 128
                        e0 = eo * 512 + off
                        # final split rides a second DGE queue so the two
                        # closing DMA chains overlap instead of serializing
                        eng = nc.scalar if (last and off) else nc.sync
                        eng.dma_start(out=out_d[q0:q0 + 128, e0:e0 + w],
                                      in_=osb)

        for kp in range(NKP):
            S_pair(0, kp)
        for kp in range(NKP):
            S_pair(1, kp)
        stage_A(0)
        stage_B(0)
        stage_O(0)
        stage_A(1)
        stage_B(1)
        stage_O(1)

    nc.compile()
    return nc


def _f8split(a):
    f8 = ml_dtypes.float8_e4m3fn
    hi = a.astype(f8)
    lo = (a - hi.astype(np.float32)).astype(f8)
    return np.ascontiguousarray(hi), np.ascontiguousarray(lo)


def _prep_inputs(x, Wq, Wk, Wv, Wo, dt_np):
    """Host-side shard + transpose + Wv/Wo fusion + fp8 hi/lo splits."""
    def dtiles3(wT, n):  # [D, n] -> [128, NDT, n], d-tile planes
        return wT.reshape(NDT, 128, n).transpose(1, 0, 2)

    # weights pre-scaled by WSC so fp8 lo-residuals stay out of subnormals;
    # the scale cancels (folded into the exp scale / the ctx 1/WSC quant)
    wq3 = np.concatenate([dtiles3(WSC * Wq.T, R), dtiles3(WSC * Wk.T, R)], axis=1)
    wqh, wql = _f8split(wq3)
    W2 = Wo.astype(np.float32) @ Wv.astype(np.float32)
    w2h, w2l = _f8split(dtiles3(WSC * W2.T, D))
    in_maps = []
    for c in range(NCORES):
        b, h = divmod(c, 2)
        xb = x[b]
        # own query half first; k-order permutation is softmax/ctx-invariant
        # (kT, E rows and xn rows all use the same permuted key order)
        xperm = np.concatenate([xb[h * SQ:(h + 1) * SQ], xb[(1 - h) * SQ:(2 - h) * SQ]], 0)
        # xt*[p, c*8+i, s] = xperm[c*512+s, i*128+p]
        xt3 = xperm.reshape(4, 512, NDT, 128).transpose(3, 0, 2, 1).reshape(128, 32, 512)
        xth, xtl = _f8split(xt3)
        # xn*[p, kt, d] = xperm[kt*128+p, d]
        xn3 = xperm.reshape(NKT, 128, D).transpose(1, 0, 2)
        xnh, xnl = _f8split(xn3)
        in_maps.append({"xth": xth, "xtl": xtl, "xnh": xnh, "xnl": xnl,
                        "wqh": wqh, "wql": wql, "w2h": w2h, "w2l": w2l})
    return in_maps


def _run(inputs, dt_np=ml_dtypes.bfloat16, trace=False, **kw):
    from concourse.bass_utils import run_bass_kernel_spmd

    key = np.dtype(dt_np).str
    if key not in _CACHE:
        _CACHE[key] = _build(dt_np)
    nc = _CACHE[key]
    in_maps = _prep_inputs(inputs["x"], inputs["Wq"], inputs["Wk"],
                           inputs["Wv"], inputs["Wo"], dt_np)
    res = run_bass_kernel_spmd(nc, in_maps, core_ids=list(range(NCORES)),
                               trace=trace, **kw)
    out = np.empty((B, S, D), np.float32)
    for c in range(NCORES):
        b, h = divmod(c, 2)
        out[b, h * SQ:(h + 1) * SQ] = res.results[c]["out"]
    return out, res


def kernel(x, mask, Wq, Wk, Wv, Wo):
    # mask is all-ones by construction (spec fill=ones) -> identity.
    out, _ = _run({"x": np.asarray(x, np.float32), "Wq": np.asarray(Wq, np.float32),
                   "Wk": np.asarray(Wk, np.float32), "Wv": np.asarray(Wv, np.float32),
                   "Wo": np.asarray(Wo, np.float32)})
    return out


# revision 31
# speedup vs baseline: 1.0345x; 1.0008x over previous
"""Low-rank self-attention on 8 trn2 NeuronCores.

reference math (per batch b):
  q = x @ Wq.T            [S,R]
  k = x @ Wk.T            [S,R]
  P = softmax(q k^T / sqrt(R))    (mask is all-ones -> no-op)
  out = (P (x @ Wv.T)) @ Wo.T = (P x) @ (Wo Wv).T      [S,D]

Key algebraic/precision moves:
  1. W2 = Wo @ Wv fused on host (f32): the v-projection disappears; the big
     context matmul contracts attention weights directly against raw x rows.
  2. Every large matmul runs in fp8(e4m3) DoubleRow mode (2 k-planes per
     instruction, 0.5 cycles/row) with an error-compensated 3-term split
        A ~ Ah + Al,  B ~ Bh + Bl   (each an exact e4m3 quantization of
        the residual),  A@B ~ Ah@Bh + Al@Bh + Ah@Bl   (Al@Bl ~ 0.1%, dropped)
     at 0.75x the bf16 cycle cost with near-bf16 accuracy.  Small-magnitude
     operands (weights, ctx) are pre-scaled by exact powers of two so the
     lo residuals stay out of the e4m3 subnormal range; the scales cancel
     algebraically (q64 k64 = 4096 qk -> folded into the exp scale;
     (ctx/64) @ (64 W2) = ctx @ W2).
  3. Scores get a constant bias shift -C before exp so E fits e4m3 range;
     softmax normalization (rowsums of Eh+El, folded to the very end)
     cancels the shift exactly.

Sharding: 8 cores = (batch b in 0..3) x (query-half h in 0..1).
Each core computes attention for its 1024 query rows over the full 2048
keys of its batch.  Host pre-stages x in both layouts as fp8 hi/lo pairs
(transposed for q/k projections, natural for the context matmul); each is
one fused dram tensor so the input side is ~20 large DMAs (HWDGE fixed
cost is 625ns per DMA, serialized).
On chip:
  q64T [128r, q] , k64T [128r, k] : 3-term DR from (wqk hi/lo, x^T hi/lo)
  scoresT[k,q] = k64T_chunk.T @ q64T -> Ebf = exp(sc*SCALE/4096 - C) (bf16)
     -> Eh = fp8(Ebf) (gpsimd copy), El = Ebf - Eh (vector sub)
  s[q] = sum_k (Eh+El)[k,q] via tiny matmuls vs ones (accum PSUM [128q,1])
  ctxX[d,q] = sum_kp DR3(xn hi/lo, E hi/lo)  (PSUM accum, 5+3 d-split)
     -> cth = fp8(ctx/64) (scalar), ctl = ctx/64 - cth (vector stt)
  out[q,eo] = sum_dp DR3(ct hi/lo, w2y hi/lo), then * (1/s[q]) per partition
Eh/El production pipelines across three engines (scalar exp, gpsimd
quantize, vector subtract) at ~1.5us/pair while pass A consumes at
~1.65us/pair, so the DoubleRow consumers are never wait-limited (a
wait-limited matmul resets the PE clock ramp).  PSUM stays within 8 banks
(score/out pool 3 + ctx pool 4 + rowsum 1; pass A's 5th accumulator
borrows from the score/out pool while it is idle).
"""

import math
import sys

import numpy as np

for _p in ("/opt/trn_rl_repo",):
    if _p not in sys.path:
        sys.path.append(_p)

import ml_dtypes  # noqa: E402

B, S, D, R = 4, 2048, 1024, 128
SQ = S // 2          # query rows per core
NCORES = 8
NDT = D // 128       # 8 d-tiles
NKT = S // 128       # 16 k-tiles
NKP = NKT // 2       # 8 k-tile pairs (DoubleRow)
NQC = SQ // 512      # 2 q-chunks per core
SCALE = 1.0 / math.sqrt(R)
CSHIFT = 5.7         # score-bias shift keeping exp() within e4m3 range
WSC = 64.0           # weight pre-scale keeping fp8 lo-residuals normal

_CACHE = {}


def _build(dt_np):
    import concourse.bass as bass  # noqa: F401
    import concourse.tile as tile
    from concourse import bacc, mybir

    DT = mybir.dt.from_np(np.dtype(dt_np))
    F8 = mybir.dt.float8e4
    F32 = mybir.dt.float32
    Exp = mybir.ActivationFunctionType.Exp
    Copy = mybir.ActivationFunctionType.Copy
    DR = mybir.MatmulPerfMode.DoubleRow
    Alu = mybir.AluOpType

    nc = bacc.Bacc(
        "TRN2", target_bir_lowering=False, debug=False,
        enable_asserts=False, num_devices=NCORES,
    )
    # fused fp8 hi/lo inputs; 3D layout puts the DoubleRow pairing dim second
    xth_d = nc.dram_tensor("xth", [128, 32, 512], F8, kind="ExternalInput").ap()
    xtl_d = nc.dram_tensor("xtl", [128, 32, 512], F8, kind="ExternalInput").ap()
    xnh_d = nc.dram_tensor("xnh", [128, NKT, D], F8, kind="ExternalInput").ap()
    xnl_d = nc.dram_tensor("xnl", [128, NKT, D], F8, kind="ExternalInput").ap()
    wqh_d = nc.dram_tensor("wqh", [128, 16, R], F8, kind="ExternalInput").ap()
    wql_d = nc.dram_tensor("wql", [128, 16, R], F8, kind="ExternalInput").ap()
    w2h_d = nc.dram_tensor("w2h", [128, NDT, D], F8, kind="ExternalInput").ap()
    w2l_d = nc.dram_tensor("w2l", [128, NDT, D], F8, kind="ExternalInput").ap()
    out_d = nc.dram_tensor("out", [SQ, D], F32, kind="ExternalOutput").ap()

    from contextlib import ExitStack

    with tile.TileContext(nc) as tc, ExitStack() as es:
        pw = es.enter_context(tc.tile_pool(name="pw", bufs=1))
        px = es.enter_context(tc.tile_pool(name="px", bufs=1))
        pqk = es.enter_context(tc.tile_pool(name="pqk", bufs=1))
        pEb = es.enter_context(tc.tile_pool(name="pEb", bufs=4))
        pE = es.enter_context(tc.tile_pool(name="pE", bufs=1))
        pct = es.enter_context(tc.tile_pool(name="pct", bufs=1))
        posb = es.enter_context(tc.tile_pool(name="posb", bufs=8))
        prs = es.enter_context(tc.tile_pool(name="prs", bufs=2))
        ps_mm = es.enter_context(tc.tile_pool(name="ps_mm", bufs=3, space="PSUM"))
        ps_ctx = es.enter_context(tc.tile_pool(name="ps_ctx", bufs=4, space="PSUM"))
        ps_s = es.enter_context(tc.tile_pool(name="ps_s", bufs=1, space="PSUM"))

        mm = nc.tensor.matmul
        cp = nc.vector.tensor_copy

        # ---- persistent inputs, one DMA queue in priority order ----------
        wqh = pw.tile([128, 16, R], F8, name="wqh")
        wql = pw.tile([128, 16, R], F8, name="wql")
        xth = px.tile([128, 32, 512], F8, name="xth")
        xtl = px.tile([128, 32, 512], F8, name="xtl")
        xnh = px.tile([128, NKT, D], F8, name="xnh")
        xnl = px.tile([128, NKT, D], F8, name="xnl")
        w2h = pw.tile([128, NDT, D], F8, name="w2h")
        w2l = pw.tile([128, NDT, D], F8, name="w2l")

        nc.sync.dma_start(out=wqh, in_=wqh_d)
        nc.sync.dma_start(out=wql, in_=wql_d)
        # chunk 0 split finely so the first projection matmuls start early
        for t, t_d in ((xth, xth_d), (xtl, xtl_d)):
            nc.sync.dma_start(out=t[:, 0:2, :], in_=t_d[:, 0:2, :])
        for lo, hi in ((2, 4), (4, 8)):
            for t, t_d in ((xth, xth_d), (xtl, xtl_d)):
                nc.sync.dma_start(out=t[:, lo:hi, :], in_=t_d[:, lo:hi, :])
        for c in range(1, 4):
            for t, t_d in ((xth, xth_d), (xtl, xtl_d)):
                nc.sync.dma_start(out=t[:, 8 * c:8 * (c + 1), :],
                                  in_=t_d[:, 8 * c:8 * (c + 1), :])
        for g in range(2):
            for t, t_d in ((xnh, xnh_d), (xnl, xnl_d)):
                nc.sync.dma_start(out=t[:, g * 8:(g + 1) * 8, :],
                                  in_=t_d[:, g * 8:(g + 1) * 8, :])
        for t, t_d in ((w2h, w2h_d), (w2l, w2l_d)):
            nc.sync.dma_start(out=t, in_=t_d)

        ones8 = pw.tile([128, 1], F8, name="ones8")
        nc.vector.memset(ones8, 1.0)
        cbias = pw.tile([128, 1], F32, name="cbias")
        nc.vector.memset(cbias, -CSHIFT)

        qTc = [pqk.tile([128, 512], DT, name=f"qT{qc}") for qc in range(NQC)]
        kTc = [pqk.tile([128, 512], DT, name=f"kT{kc}") for kc in range(4)]

        # ---- phase A: q/k projections (3-term fp8 DoubleRow) -------------
        def proj(dst, wbase, c):
            ps = ps_mm.tile([128, 512], F32, name=f"pj_{wbase}_{c}", tag="mmps")
            for p in range(4):
                wh = wqh[:, wbase + 2 * p:wbase + 2 * p + 2, :]
                wl = wql[:, wbase + 2 * p:wbase + 2 * p + 2, :]
                xh = xth[:, 8 * c + 2 * p:8 * c + 2 * p + 2, :]
                xl = xtl[:, 8 * c + 2 * p:8 * c + 2 * p + 2, :]
                st = (p == 0)
                sp = (p == 3)
                mm(ps, lhsT=wh, rhs=xh, perf_mode=DR, start=st, stop=False)
                mm(ps, lhsT=wh, rhs=xl, perf_mode=DR, start=False, stop=False)
                mm(ps, lhsT=wl, rhs=xh, perf_mode=DR, start=False, stop=sp)
            cp(dst, ps)

        proj(qTc[0], 0, 0)
        proj(kTc[0], 8, 0)
        proj(qTc[1], 0, 1)
        proj(kTc[1], 8, 1)
        proj(kTc[2], 8, 2)
        proj(kTc[3], 8, 3)

        # ---- phase B: attention, stages interleaved across q-chunks ------
        # E pair tiles [128, 2, 512]: two k-tiles stacked for DoubleRow rhs
        Ehp = [[pE.tile([128, 2, 512], F8, name=f"Eh{qc}_{kp}")
                for kp in range(NKP)] for qc in range(NQC)]
        Elp = [[pE.tile([128, 2, 512], F8, name=f"El{qc}_{kp}")
                for kp in range(NKP)] for qc in range(NQC)]
        # ctx hi/lo pair tiles [128, 2, 512]: two d-tiles stacked for DR lhsT
        Cth = [[pct.tile([128, 2, 512], F8, name=f"Ch{qc}_{dp}")
                for dp in range(4)] for qc in range(NQC)]
        Ctl = [[pct.tile([128, 2, 512], F8, name=f"Cl{qc}_{dp}")
                for dp in range(4)] for qc in range(NQC)]
        rss = [None] * NQC

        def S_pair(qc, kp):
            # scores -> Ebf=exp(.) [scalar] -> per pair: Eh=fp8(Ebf) [gpsimd
            # copy], El=Ebf-Eh [vector sub].  Three engines pipeline the
            # Eh/El production at ~1.5us/pair so the DoubleRow consumers
            # (~1.65us/pair in pass A) are never wait-limited.
            Ebf = pEb.tile([128, 2, 512], DT, name=f"Eb{qc}_{kp}", tag="Eb")
            for half in range(2):
                kt = 2 * kp + half
                sc = ps_mm.tile([128, 512], F32, name=f"sc{qc}_{kt}", tag="mmps")
                mm(sc, lhsT=kTc[kt // 4][:, (kt % 4) * 128:(kt % 4 + 1) * 128],
                   rhs=qTc[qc], start=True, stop=True)
                nc.scalar.activation(Ebf[:, half, :], sc, Exp,
                                     scale=SCALE / (WSC * WSC), bias=cbias)
            if qc == 0:
                nc.gpsimd.tensor_copy(Ehp[qc][kp], Ebf)
            else:
                # qc1's quantizes ride the scalar engine right behind their
                # exps: the Pool quant queue would otherwise hand pass A of
                # qc1 its first pair ~14us later than the scalar path does
                nc.scalar.activation(Ehp[qc][kp], Ebf, Copy)
            nc.vector.tensor_sub(Elp[qc][kp], Ebf, Ehp[qc][kp])

        def dr3(out_ps, kp, j, qc, start, stop):
            # 3-term hi/lo DoubleRow: Eh@xh + El@xh + Eh@xl
            xh = xnh[:, 2 * kp:2 * kp + 2, j * 128:(j + 1) * 128]
            xl = xnl[:, 2 * kp:2 * kp + 2, j * 128:(j + 1) * 128]
            eh = Ehp[qc][kp][:, 0:2, :]
            el = Elp[qc][kp][:, 0:2, :]
            mm(out_ps, lhsT=xh, rhs=eh, perf_mode=DR, start=start, stop=False)
            mm(out_ps, lhsT=xh, rhs=el, perf_mode=DR, start=False, stop=False)
            mm(out_ps, lhsT=xl, rhs=eh, perf_mode=DR, start=False, stop=stop)

        def ctx_quant(qc, j, cps_tile):
            # ctx psum -> fp8 hi/lo planes of the (j//2) d-pair tile.
            # high priority: these release PSUM banks and feed the out-proj;
            # without it the scheduler parks them behind the long E-production
            # queues on the scalar/vector engines.
            cth = Cth[qc][j // 2][:, j % 2, :]
            ctl = Ctl[qc][j // 2][:, j % 2, :]
            with tc.high_priority():
                nc.scalar.activation(cth, cps_tile, Copy, scale=1.0 / WSC)
                nc.vector.scalar_tensor_tensor(ctl, cps_tile, 1.0 / WSC, cth,
                                               Alu.mult, Alu.subtract)

        NJA = 5  # d-tiles in pass A: consumption ~1.65us/pair stays just
        #          above the ~1.5us/pair Eh/El production rate, so pass A
        #          finishes as production does and PE never stalls (a stall
        #          would reset the PE clock ramp).  The 5th accumulator
        #          borrows a bank from the idle score/out pool.

        def stage_A(qc):
            # ctxX d-tiles 0-4, kp-outer so E pairs are consumed as they
            # land; rowsums ride along (same E dependency, ~free matmuls).
            # one accumulation group for the whole rowsum bank: start=True
            # clears has_written for the entire bank, so only the very
            # first mm may set it; later cols overwrite-then-accumulate.
            s_ps = ps_s.tile([128, 4], F32, name=f"s_ps{qc}", tag="sps")
            cps = [ps_ctx.tile([128, 512], F32, name=f"cA{qc}_{j}", tag="ctxps")
                   for j in range(4)]
            cps.append(ps_mm.tile([128, 512], F32, name=f"cA{qc}_4", tag="mmps"))
            for kp in range(NKP):
                for j in range(NJA):
                    dr3(cps[j], kp, j, qc, start=(kp == 0), stop=(kp == NKP - 1))
                for i in range(2):
                    for j2 in range(4):
                        first = (kp == 0 and i == 0 and j2 == 0)
                        last = (kp == NKP - 1 and i == 1 and j2 == 3)
                        mm(s_ps[:, j2:j2 + 1],
                           lhsT=Ehp[qc][kp][:, i, j2 * 128:(j2 + 1) * 128],
                           rhs=ones8, start=first, stop=False)
                        mm(s_ps[:, j2:j2 + 1],
                           lhsT=Elp[qc][kp][:, i, j2 * 128:(j2 + 1) * 128],
                           rhs=ones8, start=False, stop=last)
            rs = prs.tile([128, 4], F32, name=f"rs{qc}", tag="rs")
            nc.vector.reciprocal(rs, s_ps)
            rss[qc] = rs
            for j in range(NJA):
                ctx_quant(qc, j, cps[j])

        def stage_B(qc):
            # ctxX d-tiles 5-7, j-outer (all E ready); quantize per chain so
            # banks free early for the next stage's allocations.
            for j in range(NJA, NDT):
                cpst = ps_ctx.tile([128, 512], F32, name=f"cB{qc}_{j}", tag="ctxps")
                for kp in range(NKP):
                    dr3(cpst, kp, j, qc, start=(kp == 0), stop=(kp == NKP - 1))
                ctx_quant(qc, j, cpst)

        def stage_O(qc):
            for qs in range(4):
                for eo in range(2):
                    # the very last group is split so the closing mul+DMA
                    # chain rides on a small tile (shorter tail)
                    last = (qc == NQC - 1 and qs == 3 and eo == 1)
                    for off, w in ([(0, 384), (384, 128)]
                                   if last else [(0, 512)]):
                        ops = ps_mm.tile([128, w], F32,
                                         name=f"o{qc}_{qs}_{eo}_{off}", tag="mmps")
                        qs_sl = slice(qs * 128, (qs + 1) * 128)
                        e_sl = slice(eo * 512 + off, eo * 512 + off + w)
                        for p in range(4):
                            ch = Cth[qc][p][:, 0:2, qs_sl]
                            cl = Ctl[qc][p][:, 0:2, qs_sl]
                            wh = w2h[:, 2 * p:2 * p + 2, e_sl]
                            wl = w2l[:, 2 * p:2 * p + 2, e_sl]
                            st = (p == 0)
                            sp = (p == 3)
                            mm(ops, lhsT=ch, rhs=wh, perf_mode=DR,
                               start=st, stop=False)
                            mm(ops, lhsT=cl, rhs=wh, perf_mode=DR,
                               start=False, stop=False)
                            mm(ops, lhsT=ch, rhs=wl, perf_mode=DR,
                               start=False, stop=sp)
                        osb = posb.tile([128, w], F32,
                                        name=f"osb{qc}_{qs}_{eo}_{off}", tag="osb")
                        nc.scalar.mul(osb, ops, rss[qc][:, qs:qs + 1])
                        q0 = qc * 512 + qs * 128
                        e0 = eo * 512 + off
                        # final split rides a second DGE queue so the two
                        # closing DMA chains overlap instead of serializing
                        eng = nc.scalar if (last and not off) else nc.sync
                        eng.dma_start(out=out_d[q0:q0 + 128, e0:e0 + w],
                                      in_=osb)

        for kp in range(NKP):
            S_pair(0, kp)
        for kp in range(NKP):
            S_pair(1, kp)
        stage_A(0)
        stage_B(0)
        stage_O(0)
        stage_A(1)
        stage_B(1)
        stage_O(1)

    nc.compile()
    return nc


def _f8split(a):
    f8 = ml_dtypes.float8_e4m3fn
    hi = a.astype(f8)
    lo = (a - hi.astype(np.float32)).astype(f8)
    return np.ascontiguousarray(hi), np.ascontiguousarray(lo)


def _prep_inputs(x, Wq, Wk, Wv, Wo, dt_np):
    """Host-side shard + transpose + Wv/Wo fusion + fp8 hi/lo splits."""
    def dtiles3(wT, n):  # [D, n] -> [128, NDT, n], d-tile planes
        return wT.reshape(NDT, 128, n).transpose(1, 0, 2)

    # weights pre-scaled by WSC so fp8 lo-residuals stay out of subnormals;
    # the scale cancels (folded into the exp scale / the ctx 1/WSC quant)
    wq3 = np.concatenate([dtiles3(WSC * Wq.T, R), dtiles3(WSC * Wk.T, R)], axis=1)
    wqh, wql = _f8split(wq3)
    W2 = Wo.astype(np.float32) @ Wv.astype(np.float32)
    w2h, w2l = _f8split(dtiles3(WSC * W2.T, D))
    in_maps = []
    for c in range(NCORES):
        b, h = divmod(c, 2)
        xb = x[b]
        # own query half first; k-order permutation is softmax/ctx-invariant
        # (kT, E rows and xn rows all use the same permuted key order)
        xperm = np.concatenate([xb[h * SQ:(h + 1) * SQ], xb[(1 - h) * SQ:(2 - h) * SQ]], 0)
        # xt*[p, c*8+i, s] = xperm[c*512+s, i*128+p]
        xt3 = xperm.reshape(4, 512, NDT, 128).transpose(3, 0, 2, 1).reshape(128, 32, 512)
        xth, xtl = _f8split(xt3)
        # xn*[p, kt, d] = xperm[kt*128+p, d]
        xn3 = xperm.reshape(NKT, 128, D).transpose(1, 0, 2)
        xnh, xnl = _f8split(xn3)
        in_maps.append({"xth": xth, "xtl": xtl, "xnh": xnh, "xnl": xnl,
                        "wqh": wqh, "wql": wql, "w2h": w2h, "w2l": w2l})
    return in_maps


def _run(inputs, dt_np=ml_dtypes.bfloat16, trace=False, **kw):
    from concourse.bass_utils import run_bass_kernel_spmd

    key = np.dtype(dt_np).str
    if key not in _CACHE:
        _CACHE[key] = _build(dt_np)
    nc = _CACHE[key]
    in_maps = _prep_inputs(inputs["x"], inputs["Wq"], inputs["Wk"],
                           inputs["Wv"], inputs["Wo"], dt_np)
    res = run_bass_kernel_spmd(nc, in_maps, core_ids=list(range(NCORES)),
                               trace=trace, **kw)
    out = np.empty((B, S, D), np.float32)
    for c in range(NCORES):
        b, h = divmod(c, 2)
        out[b, h * SQ:(h + 1) * SQ] = res.results[c]["out"]
    return out, res


def kernel(x, mask, Wq, Wk, Wv, Wo):
    # mask is all-ones by construction (spec fill=ones) -> identity.
    out, _ = _run({"x": np.asarray(x, np.float32), "Wq": np.asarray(Wq, np.float32),
                   "Wk": np.asarray(Wk, np.float32), "Wv": np.asarray(Wv, np.float32),
                   "Wo": np.asarray(Wo, np.float32)})
    return out
